# revision 11
# baseline (speedup 1.0000x reference)
"""Dense causal transformer attention block on 8 Trainium2 NeuronCores.

Problem: out = CausalAttention(RoPE(x@wq, x@wk), x@wv) @ wo
  x [2, 4096, 2048], 16 heads x 128 dim, fp32 I/O.

Sharding: tensor-parallel over heads. Core c owns heads {2c, 2c+1}:
  - computes qT/kT/vT ([head_dim, seq] layout) for its heads from the full
    (host-pre-transposed) xT, RoPE applied on-chip, V re-transposed to
    [seq, head_dim] on the PE (identity-matmul transpose),
  - runs causal attention in transposed form (scoresT = k @ qT so the
    softmax weights come out as the moving operand of the A@V matmul —
    no on-chip transpose of the probability matrix needed),
  - denominators via an all-ones [128,128] stationary matmul (comes out
    pre-broadcast across partitions),
  - computes its partial output projection o_local @ wo[rows of its heads].
Host sums the 8 partial outputs (the wo row-parallel all-reduce).

Compute dtype bf16 (PE 1 cycle/row), accumulation fp32 in PSUM.
"""
import sys

for _p in ("/opt/trn_rl_repo",):
    if _p not in sys.path:
        sys.path.insert(0, _p)

import numpy as np
import ml_dtypes
from contextlib import ExitStack

import concourse.bass as bass
import concourse.tile as tile
from concourse import bacc, mybir
from concourse import bass_utils

B, S, D = 2, 4096, 2048
H, DH = 16, 128
HALF = DH // 2
NC = 8
HPC = H // NC          # heads per core = 2
DOUT = HPC * DH        # 256 local proj width
ROPE_BASE = 10000.0
SCALE = 1.0 / float(np.sqrt(DH))
SQ = 512               # query tile (free dim of scoresT)
SKB = 128              # key block (partitions of scoresT)
KM = D // 128          # 16 contraction blocks
NSQ = S // SQ          # 8 query tiles per batch
BF = mybir.dt.bfloat16
F32 = mybir.dt.float32

_CACHED = {}


def _build():
    nc = bacc.Bacc("TRN2", target_bir_lowering=False, debug=False, num_devices=NC)

    xT = nc.dram_tensor("xT", [D, B * S], BF, kind="ExternalInput").ap()
    wq = nc.dram_tensor("wq", [D, DOUT], BF, kind="ExternalInput").ap()
    wk = nc.dram_tensor("wk", [D, DOUT], BF, kind="ExternalInput").ap()
    wv = nc.dram_tensor("wv", [D, DOUT], BF, kind="ExternalInput").ap()
    wo = nc.dram_tensor("wo", [DOUT, D], BF, kind="ExternalInput").ap()
    cosf = nc.dram_tensor("cosf", [DH, S], F32, kind="ExternalInput").ap()
    sins = nc.dram_tensor("sins", [DH, S], F32, kind="ExternalInput").ap()
    masks = nc.dram_tensor("masks", [SKB, 4 * SQ], BF, kind="ExternalInput").ap()
    ones = nc.dram_tensor("ones", [128, 128], BF, kind="ExternalInput").ap()
    ident = nc.dram_tensor("ident", [128, 128], BF, kind="ExternalInput").ap()
    outp = nc.dram_tensor("outp", [B * S, D], BF, kind="ExternalOutput").ap()

    with tile.TileContext(nc) as tc, ExitStack() as ctx:
        const = ctx.enter_context(tc.tile_pool(name="const", bufs=1))
        xpool = ctx.enter_context(tc.tile_pool(name="xpool", bufs=18))
        qkv = ctx.enter_context(tc.tile_pool(name="qkv", bufs=1))
        rope = ctx.enter_context(tc.tile_pool(name="rope", bufs=2))
        attn = ctx.enter_context(tc.tile_pool(name="attn", bufs=4))
        opool = ctx.enter_context(tc.tile_pool(name="opool", bufs=4))

        # ---- persistent constants -------------------------------------
        # Per-km weight tiles so the first matmul only waits on 1/16th of
        # the weight DMA traffic.
        wq_sb = [const.tile([128, DOUT], BF, name=f"wq_sb{km}") for km in range(KM)]
        wk_sb = [const.tile([128, DOUT], BF, name=f"wk_sb{km}") for km in range(KM)]
        wv_sb = [const.tile([128, DOUT], BF, name=f"wv_sb{km}") for km in range(KM)]
        for km in range(KM):
            for w_ap, w_sb in ((wq, wq_sb), (wk, wk_sb), (wv, wv_sb)):
                nc.sync.dma_start(
                    w_sb[km][:], w_ap[km * 128:(km + 1) * 128, :])
        cos_sb = const.tile([DH, S], F32, name="cos_sb")
        sin_sb = const.tile([DH, S], F32, name="sin_sb")  # rows 0-63 = -sin
        nc.sync.dma_start(cos_sb[:], cosf[:])
        nc.sync.dma_start(sin_sb[:], sins[:])
        mask_sb = const.tile([SKB, 4 * SQ], BF, name="mask_sb")
        nc.sync.dma_start(mask_sb[:], masks[:])
        ones_sb = const.tile([128, 128], BF, name="ones_sb")
        nc.sync.dma_start(ones_sb[:], ones[:])
        id_sb = const.tile([128, 128], BF, name="id_sb")
        nc.sync.dma_start(id_sb[:], ident[:])
        wo_sb = const.tile([128, HPC * D], BF, name="wo_sb")    # [p, h*2048+n]
        nc.sync.dma_start(
            wo_sb[:].rearrange("p (a n) -> p a n", n=D),
            wo.rearrange("(a p) n -> p a n", p=128),
        )

        qT = [qkv.tile([128, S], BF, tag=f"qT{j}", name=f"qT{j}") for j in range(HPC)]
        kT = [qkv.tile([128, S], BF, tag=f"kT{j}", name=f"kT{j}") for j in range(HPC)]
        vsb = [qkv.tile([128, S], BF, tag=f"v{j}", name=f"v{j}") for j in range(HPC)]
        oT = [qkv.tile([128, S], BF, tag=f"oT{j}", name=f"oT{j}") for j in range(HPC)]

        for b in range(B):
            # ---- projections + RoPE for batch b ------------------------
            # qT/kT/vT [dh, seq] per head; V re-transposed via PE.
            with tc.tile_pool(name=f"psp{b}", bufs=1, space="PSUM") as psp:
                for t in range(NSQ):
                    s0 = t * SQ
                    xb = [xpool.tile([128, SQ], BF, tag="xb", name=f"xb{km}")
                          for km in range(KM)]
                    for km in range(KM):
                        nc.sync.dma_start(
                            xb[km][:],
                            xT[km * 128:(km + 1) * 128, b * S + s0: b * S + s0 + SQ])
                    pq = [psp.tile([128, SQ], F32, tag=f"pq{j}", name=f"pq{j}")
                          for j in range(HPC)]
                    pk = [psp.tile([128, SQ], F32, tag=f"pk{j}", name=f"pk{j}")
                          for j in range(HPC)]
                    pv = [psp.tile([128, SQ], F32, tag=f"pv{j}", name=f"pv{j}")
                          for j in range(HPC)]
                    for km in range(KM):
                        st, sp = km == 0, km == KM - 1
                        for j in range(HPC):
                            nc.tensor.matmul(
                                pq[j][:], wq_sb[km][:, j * DH:(j + 1) * DH],
                                xb[km][:], start=st, stop=sp)
                            nc.tensor.matmul(
                                pk[j][:], wk_sb[km][:, j * DH:(j + 1) * DH],
                                xb[km][:], start=st, stop=sp)
                            nc.tensor.matmul(
                                pv[j][:], wv_sb[km][:, j * DH:(j + 1) * DH],
                                xb[km][:], start=st, stop=sp)
                    # RoPE: dst = p*cos + rot(p)*sin_signed (rot = half swap)
                    for j in range(HPC):
                        for ps, dstt in ((pq[j], qT[j]), (pk[j], kT[j])):
                            rt = rope.tile([128, SQ], F32, tag="rot", name="rt")
                            nc.scalar.copy(rt[0:HALF, :], ps[HALF:128, :])
                            nc.scalar.copy(rt[HALF:128, :], ps[0:HALF, :])
                            m1 = rope.tile([128, SQ], F32, tag="m1", name="m1")
                            nc.vector.tensor_mul(m1[:], ps[:], cos_sb[:, s0:s0 + SQ])
                            nc.vector.tensor_mul(rt[:], rt[:], sin_sb[:, s0:s0 + SQ])
                            nc.vector.tensor_add(dstt[:, s0:s0 + SQ], m1[:], rt[:])
                        # V: copy vT psum to sbuf (ACT), then PE-transpose
                        # each 128-block back to [seq, dh] layout.
                        vt = rope.tile([128, SQ], BF, tag="vt", name="vt")
                        nc.scalar.copy(vt[:], pv[j][:])
                        for sub in range(4):
                            ptr = psp.tile([128, 128], BF, tag="ptr", bufs=2,
                                           name="ptr")
                            nc.tensor.transpose(
                                ptr[:], vt[:, sub * 128:(sub + 1) * 128], id_sb[:])
                            nc.vector.tensor_copy(
                                vsb[j][:, (4 * t + sub) * 128:(4 * t + sub + 1) * 128],
                                ptr[:])

            # ---- causal attention per head -----------------------------
            # pscr is a 2-bank [128,1024] tile covering a PAIR of key
            # blocks: one exp instruction per pair (ACT is the attention
            # rate limiter).  Denominators quad-batched: DVE pre-sums 4
            # blocks of exp weights, one ones-matmul per quad.
            with tc.tile_pool(name=f"psa{b}", bufs=1, space="PSUM") as psa:
                for j in range(HPC):
                    for t in range(NSQ):
                        s0 = t * SQ
                        nblk = 4 * t + 4
                        npair = nblk // 2
                        nquad = npair // 2
                        po = psa.tile([128, SQ], F32, tag="po", bufs=2, name="po")
                        pd = psa.tile([128, SQ], F32, tag="pd", bufs=2, name="pd")
                        prev_et = None
                        for p in range(npair):
                            pscr = psa.tile([128, 2 * SQ], F32, tag="pscr",
                                            bufs=2, name="pscr")
                            for h in range(2):
                                u = 2 * p + h
                                nc.tensor.matmul(
                                    pscr[:, h * SQ:(h + 1) * SQ],
                                    kT[j][:, u * SKB:(u + 1) * SKB],
                                    qT[j][:, s0:s0 + SQ], start=True, stop=True,
                                    skip_group_check=True)
                            et = attn.tile([128, 2 * SQ], BF, tag="et", bufs=4,
                                           name="et")
                            nc.scalar.activation(
                                et[:], pscr[:], mybir.ActivationFunctionType.Exp,
                                scale=SCALE)
                            if 2 * p >= 4 * t:  # pair on the diagonal band
                                r = 2 * p - 4 * t   # 0 or 2
                                nc.vector.tensor_mul(
                                    et[:], et[:],
                                    mask_sb[:, r * SQ:(r + 2) * SQ])
                            for h in range(2):
                                u = 2 * p + h
                                nc.tensor.matmul(
                                    po[:], vsb[j][:, u * 128:(u + 1) * 128],
                                    et[:, h * SQ:(h + 1) * SQ],
                                    start=u == 0, stop=u == nblk - 1)
                            if p % 2 == 1:
                                # quad-batched denominator: DVE pre-sums 4
                                # blocks, one ones-matmul per quad.
                                qi = p // 2
                                qs = attn.tile([128, 2 * SQ], BF, tag="qs",
                                               bufs=2, name="qs")
                                nc.vector.tensor_add(qs[:], prev_et[:], et[:])
                                qs2 = attn.tile([128, SQ], BF, tag="qs2",
                                                bufs=2, name="qs2")
                                nc.vector.tensor_add(
                                    qs2[:], qs[:, 0:SQ], qs[:, SQ:2 * SQ])
                                nc.tensor.matmul(
                                    pd[:], ones_sb[:], qs2[:],
                                    start=qi == 0, stop=qi == nquad - 1)
                            prev_et = et
                        rec = attn.tile([128, SQ], F32, tag="rec", bufs=2,
                                        name="rec")
                        nc.vector.reciprocal(rec[:], pd[:])
                        nc.vector.tensor_mul(oT[j][:, s0:s0 + SQ], po[:], rec[:])

            # ---- partial output projection -----------------------------
            # psum->sbuf copies alternate DVE/ACT to balance engine load.
            with tc.tile_pool(name=f"psf{b}", bufs=2, space="PSUM") as psf:
                for m in range(S // 128):
                    for n in range(D // 512):
                        pf = psf.tile([128, 512], F32, tag="pf", name="pf")
                        for jj in range(HPC):
                            nc.tensor.matmul(
                                pf[:], oT[jj][:, m * 128:(m + 1) * 128],
                                wo_sb[:, jj * D + n * 512:
                                      jj * D + (n + 1) * 512],
                                start=jj == 0, stop=jj == HPC - 1)
                        ob = opool.tile([128, 512], BF, tag="ob", name="ob")
                        if (m + n) % 2 == 0:
                            nc.vector.tensor_copy(ob[:], pf[:])
                        else:
                            nc.scalar.copy(ob[:], pf[:])
                        nc.sync.dma_start(
                            outp[b * S + m * 128: b * S + (m + 1) * 128,
                                 n * 512:(n + 1) * 512], ob[:])

    nc.compile()
    return nc


def _host_inputs(x, wq, wk, wv, wo):
    bf16 = ml_dtypes.bfloat16
    xT = np.ascontiguousarray(x.reshape(B * S, D).T).astype(bf16)

    half = HALF
    inv = 1.0 / (ROPE_BASE ** (np.arange(0, half, dtype=np.float32) / half))
    ang = np.arange(S, dtype=np.float32)[:, None] * inv[None, :]
    cos = np.cos(ang).astype(np.float32)
    sin = np.sin(ang).astype(np.float32)
    cosf = np.ascontiguousarray(
        np.concatenate([cos, cos], axis=1).T)      # [128, S]
    sins = np.concatenate([-sin, sin], axis=1).T   # rows 0-63 negated
    sins = np.ascontiguousarray(sins)

    i = np.arange(SKB)[:, None]
    jj = np.arange(SQ)[None, :]
    masks = np.concatenate(
        [(i + r * SKB <= jj) for r in range(4)], axis=1).astype(bf16)
    ones = np.ones((128, 128), dtype=bf16)
    ident = np.eye(128, dtype=bf16)

    in_maps = []
    for c in range(NC):
        lo = c * DOUT
        in_maps.append({
            "xT": xT,
            "wq": np.ascontiguousarray(wq[:, lo:lo + DOUT]).astype(bf16),
            "wk": np.ascontiguousarray(wk[:, lo:lo + DOUT]).astype(bf16),
            "wv": np.ascontiguousarray(wv[:, lo:lo + DOUT]).astype(bf16),
            "wo": np.ascontiguousarray(wo[lo:lo + DOUT, :]).astype(bf16),
            "cosf": cosf,
            "sins": sins,
            "masks": masks,
            "ones": ones,
            "ident": ident,
        })
    return in_maps


def kernel(x, wq, wk, wv, wo, cos, sin, _trace=False, _tmpdir=None):
    if "nc" not in _CACHED:
        _CACHED["nc"] = _build()
    nc = _CACHED["nc"]
    in_maps = _host_inputs(
        np.asarray(x, dtype=np.float32), np.asarray(wq, dtype=np.float32),
        np.asarray(wk, dtype=np.float32), np.asarray(wv, dtype=np.float32),
        np.asarray(wo, dtype=np.float32))
    res = bass_utils.run_bass_kernel_spmd(
        nc, in_maps, core_ids=list(range(NC)), trace=_trace, tmpdir=_tmpdir)
    acc = np.zeros((B * S, D), dtype=np.float32)
    for c in range(NC):
        acc += res.results[c]["outp"].astype(np.float32)
    out = acc.reshape(B, S, D)
    if _trace:
        _CACHED["last_results"] = res
    return out


# revision 12
# speedup vs baseline: 1.1266x; 1.1266x over previous
"""Dense causal transformer attention block on 8 Trainium2 NeuronCores.

Problem: out = CausalAttention(RoPE(x@wq, x@wk), x@wv) @ wo
  x [2, 4096, 2048], 16 heads x 128 dim, fp32 I/O.

Sharding: tensor-parallel over heads. Core c owns heads {2c, 2c+1}:
  - computes qT/kT/vT ([head_dim, seq] layout) for its heads from the full
    (host-pre-transposed) xT, RoPE applied on-chip, V re-transposed to
    [seq, head_dim] on the PE (identity-matmul transpose),
  - runs causal attention in transposed form (scoresT = k @ qT so the
    softmax weights come out as the moving operand of the A@V matmul —
    no on-chip transpose of the probability matrix needed),
  - denominators via an all-ones [128,128] stationary matmul (comes out
    pre-broadcast across partitions),
  - computes its partial output projection o_local @ wo[rows of its heads].
Host sums the 8 partial outputs (the wo row-parallel all-reduce).

Compute dtype bf16 (PE 1 cycle/row), accumulation fp32 in PSUM.
"""
import sys

for _p in ("/opt/trn_rl_repo",):
    if _p not in sys.path:
        sys.path.insert(0, _p)

import numpy as np
import ml_dtypes
from contextlib import ExitStack

import concourse.bass as bass
import concourse.tile as tile
from concourse import bacc, mybir
from concourse import bass_utils

B, S, D = 2, 4096, 2048
H, DH = 16, 128
HALF = DH // 2
NC = 8
HPC = H // NC          # heads per core = 2
DOUT = HPC * DH        # 256 local proj width
ROPE_BASE = 10000.0
SCALE = 1.0 / float(np.sqrt(DH))
SQ = 512               # query tile (free dim of scoresT)
SKB = 128              # key block (partitions of scoresT)
KM = D // 128          # 16 contraction blocks
NSQ = S // SQ          # 8 query tiles per batch
BF = mybir.dt.bfloat16
F32 = mybir.dt.float32

_CACHED = {}


def _build():
    nc = bacc.Bacc("TRN2", target_bir_lowering=False, debug=False, num_devices=NC)

    xT = nc.dram_tensor("xT", [D, B * S], BF, kind="ExternalInput").ap()
    wq = nc.dram_tensor("wq", [D, DOUT], BF, kind="ExternalInput").ap()
    wk = nc.dram_tensor("wk", [D, DOUT], BF, kind="ExternalInput").ap()
    wv = nc.dram_tensor("wv", [D, DOUT], BF, kind="ExternalInput").ap()
    wo = nc.dram_tensor("wo", [DOUT, D], BF, kind="ExternalInput").ap()
    cosf = nc.dram_tensor("cosf", [DH, S], F32, kind="ExternalInput").ap()
    sins = nc.dram_tensor("sins", [DH, S], F32, kind="ExternalInput").ap()
    masks = nc.dram_tensor("masks", [SKB, 4 * SQ], BF, kind="ExternalInput").ap()
    ones = nc.dram_tensor("ones", [128, 128], BF, kind="ExternalInput").ap()
    ident = nc.dram_tensor("ident", [128, 128], BF, kind="ExternalInput").ap()
    outp = nc.dram_tensor("outp", [B * S, D], BF, kind="ExternalOutput").ap()

    with tile.TileContext(nc) as tc, ExitStack() as ctx:
        const = ctx.enter_context(tc.tile_pool(name="const", bufs=1))
        xpool = ctx.enter_context(tc.tile_pool(name="xpool", bufs=18))
        qkv = ctx.enter_context(tc.tile_pool(name="qkv", bufs=1))
        rope = ctx.enter_context(tc.tile_pool(name="rope", bufs=2))
        attn = ctx.enter_context(tc.tile_pool(name="attn", bufs=4))
        opool = ctx.enter_context(tc.tile_pool(name="opool", bufs=4))

        # ---- persistent constants -------------------------------------
        # Per-km weight tiles so the first matmul only waits on 1/16th of
        # the weight DMA traffic.
        wq_sb = [const.tile([128, DOUT], BF, name=f"wq_sb{km}") for km in range(KM)]
        wk_sb = [const.tile([128, DOUT], BF, name=f"wk_sb{km}") for km in range(KM)]
        wv_sb = [const.tile([128, DOUT], BF, name=f"wv_sb{km}") for km in range(KM)]
        for km in range(KM):
            for w_ap, w_sb in ((wq, wq_sb), (wk, wk_sb), (wv, wv_sb)):
                nc.sync.dma_start(
                    w_sb[km][:], w_ap[km * 128:(km + 1) * 128, :])
        cos_sb = const.tile([DH, S], F32, name="cos_sb")
        sin_sb = const.tile([DH, S], F32, name="sin_sb")  # rows 0-63 = -sin
        nc.sync.dma_start(cos_sb[:], cosf[:])
        nc.sync.dma_start(sin_sb[:], sins[:])
        mask_sb = const.tile([SKB, 4 * SQ], BF, name="mask_sb")
        nc.sync.dma_start(mask_sb[:], masks[:])
        ones_sb = const.tile([128, 128], BF, name="ones_sb")
        nc.sync.dma_start(ones_sb[:], ones[:])
        id_sb = const.tile([128, 128], BF, name="id_sb")
        nc.sync.dma_start(id_sb[:], ident[:])
        wo_sb = const.tile([128, HPC * D], BF, name="wo_sb")    # [p, h*2048+n]
        nc.sync.dma_start(
            wo_sb[:].rearrange("p (a n) -> p a n", n=D),
            wo.rearrange("(a p) n -> p a n", p=128),
        )

        qT = [qkv.tile([128, S], BF, tag=f"qT{j}", name=f"qT{j}") for j in range(HPC)]
        kT = [qkv.tile([128, S], BF, tag=f"kT{j}", name=f"kT{j}") for j in range(HPC)]
        vsb = [qkv.tile([128, S], BF, tag=f"v{j}", name=f"v{j}") for j in range(HPC)]
        oT = [qkv.tile([128, S], BF, tag=f"oT{j}", name=f"oT{j}") for j in range(HPC)]

        for b in range(B):
            # ---- fully merged per-t pipeline ---------------------------
            # One 8-bank PSUM pool per batch:
            #   pqk (1 bank)   q then k accumulation, per head, sequential
            #   pv  (1 bank)   v accumulation + PE-transpose targets
            #   pscr(2x2 bank) attention score pairs [128,1024]
            #   po  (1 bank)   A@V accumulator
            #   pd  (1 bank)   denominator accumulator
            # Out-proj pf tiles share the pscr tag's slots.
            with tc.tile_pool(name=f"ps{b}", bufs=1, space="PSUM") as psm:
                for t in range(NSQ):
                    s0 = t * SQ
                    xb = [xpool.tile([128, SQ], BF, tag="xb", name=f"xb{km}")
                          for km in range(KM)]
                    for km in range(KM):
                        nc.sync.dma_start(
                            xb[km][:],
                            xT[km * 128:(km + 1) * 128, b * S + s0: b * S + s0 + SQ])
                    # --- projections + RoPE, head by head ---------------
                    for j in range(HPC):
                        for w_sb, dstt in ((wq_sb, qT[j]), (wk_sb, kT[j])):
                            pp = psm.tile([128, SQ], F32, tag="pqk", name="pp")
                            for km in range(KM):
                                nc.tensor.matmul(
                                    pp[:], w_sb[km][:, j * DH:(j + 1) * DH],
                                    xb[km][:], start=km == 0, stop=km == KM - 1)
                            rt = rope.tile([128, SQ], F32, tag="rot", name="rt")
                            nc.vector.tensor_copy(rt[0:HALF, :], pp[HALF:128, :])
                            nc.vector.tensor_copy(rt[HALF:128, :], pp[0:HALF, :])
                            m1 = rope.tile([128, SQ], F32, tag="m1", name="m1")
                            nc.vector.tensor_mul(m1[:], pp[:], cos_sb[:, s0:s0 + SQ])
                            nc.vector.tensor_mul(rt[:], rt[:], sin_sb[:, s0:s0 + SQ])
                            nc.vector.tensor_add(dstt[:, s0:s0 + SQ], m1[:], rt[:])
                        pv = psm.tile([128, SQ], F32, tag="pv", name="pv")
                        for km in range(KM):
                            nc.tensor.matmul(
                                pv[:], wv_sb[km][:, j * DH:(j + 1) * DH],
                                xb[km][:], start=km == 0, stop=km == KM - 1)
                        vt = rope.tile([128, SQ], BF, tag="vt", name="vt")
                        nc.scalar.copy(vt[:], pv[:])
                        for sub in range(4):
                            ptr = psm.tile([128, 128], BF, tag="pv", name="ptr")
                            nc.tensor.transpose(
                                ptr[:], vt[:, sub * 128:(sub + 1) * 128], id_sb[:])
                            nc.vector.tensor_copy(
                                vsb[j][:, (4 * t + sub) * 128:(4 * t + sub + 1) * 128],
                                ptr[:])
                    # --- causal attention for this query tile -----------
                    for j in range(HPC):
                        nblk = 4 * t + 4
                        npair = nblk // 2
                        nquad = npair // 2
                        po = psm.tile([128, SQ], F32, tag="po", name="po")
                        pd = psm.tile([128, SQ], F32, tag="pd", name="pd")
                        prev_et = None
                        for p in range(npair):
                            pscr = psm.tile([128, 2 * SQ], F32, tag="pscr",
                                            bufs=2, name="pscr")
                            for h in range(2):
                                u = 2 * p + h
                                nc.tensor.matmul(
                                    pscr[:, h * SQ:(h + 1) * SQ],
                                    kT[j][:, u * SKB:(u + 1) * SKB],
                                    qT[j][:, s0:s0 + SQ], start=True, stop=True,
                                    skip_group_check=True)
                            et = attn.tile([128, 2 * SQ], BF, tag="et", bufs=4,
                                           name="et")
                            nc.scalar.activation(
                                et[:], pscr[:], mybir.ActivationFunctionType.Exp,
                                scale=SCALE)
                            if 2 * p >= 4 * t:  # pair on the diagonal band
                                r = 2 * p - 4 * t   # 0 or 2
                                nc.vector.tensor_mul(
                                    et[:], et[:],
                                    mask_sb[:, r * SQ:(r + 2) * SQ])
                            for h in range(2):
                                u = 2 * p + h
                                nc.tensor.matmul(
                                    po[:], vsb[j][:, u * 128:(u + 1) * 128],
                                    et[:, h * SQ:(h + 1) * SQ],
                                    start=u == 0, stop=u == nblk - 1)
                            if p % 2 == 1:
                                qi = p // 2
                                qs = attn.tile([128, 2 * SQ], BF, tag="qs",
                                               bufs=2, name="qs")
                                nc.vector.tensor_add(qs[:], prev_et[:], et[:])
                                qs2 = attn.tile([128, SQ], BF, tag="qs2",
                                                bufs=2, name="qs2")
                                nc.vector.tensor_add(
                                    qs2[:], qs[:, 0:SQ], qs[:, SQ:2 * SQ])
                                nc.tensor.matmul(
                                    pd[:], ones_sb[:], qs2[:],
                                    start=qi == 0, stop=qi == nquad - 1)
                            prev_et = et
                        rec = attn.tile([128, SQ], F32, tag="rec", bufs=2,
                                        name="rec")
                        nc.vector.reciprocal(rec[:], pd[:])
                        nc.vector.tensor_mul(oT[j][:, s0:s0 + SQ], po[:], rec[:])
                    # --- out-proj for the 4 seq blocks completed at t ----
                    # pf tiles borrow the pscr tag's 2-bank slots.
                    for m in range(4 * t, 4 * t + 4):
                        for n in range(D // 512):
                            pf = psm.tile([128, 512], F32, tag="pscr", bufs=2,
                                          name="pf")
                            for jj in range(HPC):
                                nc.tensor.matmul(
                                    pf[:], oT[jj][:, m * 128:(m + 1) * 128],
                                    wo_sb[:, jj * D + n * 512:
                                          jj * D + (n + 1) * 512],
                                    start=jj == 0, stop=jj == HPC - 1)
                            ob = opool.tile([128, 512], BF, tag="ob", name="ob")
                            if (m + n) % 2 == 0:
                                nc.vector.tensor_copy(ob[:], pf[:])
                            else:
                                nc.scalar.copy(ob[:], pf[:])
                            nc.sync.dma_start(
                                outp[b * S + m * 128: b * S + (m + 1) * 128,
                                     n * 512:(n + 1) * 512], ob[:])

    nc.compile()
    return nc


def _host_inputs(x, wq, wk, wv, wo):
    bf16 = ml_dtypes.bfloat16
    xT = np.ascontiguousarray(x.reshape(B * S, D).T).astype(bf16)

    half = HALF
    inv = 1.0 / (ROPE_BASE ** (np.arange(0, half, dtype=np.float32) / half))
    ang = np.arange(S, dtype=np.float32)[:, None] * inv[None, :]
    cos = np.cos(ang).astype(np.float32)
    sin = np.sin(ang).astype(np.float32)
    cosf = np.ascontiguousarray(
        np.concatenate([cos, cos], axis=1).T)      # [128, S]
    sins = np.concatenate([-sin, sin], axis=1).T   # rows 0-63 negated
    sins = np.ascontiguousarray(sins)

    i = np.arange(SKB)[:, None]
    jj = np.arange(SQ)[None, :]
    masks = np.concatenate(
        [(i + r * SKB <= jj) for r in range(4)], axis=1).astype(bf16)
    ones = np.ones((128, 128), dtype=bf16)
    ident = np.eye(128, dtype=bf16)

    in_maps = []
    for c in range(NC):
        lo = c * DOUT
        in_maps.append({
            "xT": xT,
            "wq": np.ascontiguousarray(wq[:, lo:lo + DOUT]).astype(bf16),
            "wk": np.ascontiguousarray(wk[:, lo:lo + DOUT]).astype(bf16),
            "wv": np.ascontiguousarray(wv[:, lo:lo + DOUT]).astype(bf16),
            "wo": np.ascontiguousarray(wo[lo:lo + DOUT, :]).astype(bf16),
            "cosf": cosf,
            "sins": sins,
            "masks": masks,
            "ones": ones,
            "ident": ident,
        })
    return in_maps


def kernel(x, wq, wk, wv, wo, cos, sin, _trace=False, _tmpdir=None):
    if "nc" not in _CACHED:
        _CACHED["nc"] = _build()
    nc = _CACHED["nc"]
    in_maps = _host_inputs(
        np.asarray(x, dtype=np.float32), np.asarray(wq, dtype=np.float32),
        np.asarray(wk, dtype=np.float32), np.asarray(wv, dtype=np.float32),
        np.asarray(wo, dtype=np.float32))
    res = bass_utils.run_bass_kernel_spmd(
        nc, in_maps, core_ids=list(range(NC)), trace=_trace, tmpdir=_tmpdir)
    acc = np.zeros((B * S, D), dtype=np.float32)
    for c in range(NC):
        acc += res.results[c]["outp"].astype(np.float32)
    out = acc.reshape(B, S, D)
    if _trace:
        _CACHED["last_results"] = res
    return out
